# revision 15
# baseline (speedup 1.0000x reference)
"""Trainium2 Bass kernel for the L2RU linear recurrent unit.

Strategy
--------
Host: the tiny 256x256 parameter computation (Cayley transform, Cholesky,
solves) runs in float64 numpy, producing A, B, D.  All data is pre-
transposed to feature-major layout on the host and pre-rounded to f32r
(11-mantissa-bit fp32, the TensorE full-rate fp32 mode).

Device (per core, data-parallel over batch: 4 rows/core x 8 cores):
  time is split into C=128 chunks of L=16 steps.
  P1  V^T = B @ U^T              (big matmul, v_t placed at position t+1)
  P2  local scans: 15 sequential matmul steps, each over all 128 chunks
      at once (free dim 512) -> chunk-local states, in place over V.
  P3  Kogge-Stone carry propagation over the 128 chunk-final states
      (7 matmul passes with A^(16*2^s)).
  P4  fix-up: for j=1..16 add carry @ A^j to every chunk's local states.
  P5  outputs O^T = C @ X^T + D @ U^T, streamed out per 512-column block.

All matmuls run in f32r (free dim >= 256 -> full PE rate), fp32 PSUM
accumulation.  Verified absmax-relative error vs the fp32 reference
~2e-4 (the recurrence is contractive: spectral radius of A ~0.948).
"""
import sys

sys.path.insert(0, "/opt/trn_rl_repo")

import numpy as np

import concourse.bacc as bacc
import concourse.tile as tile
import concourse.mybir as mybir
from concourse.bass_utils import run_bass_kernel_spmd

N = 256
BATCH = 32
SEQ = 2048
NCORES = 8
BPC = BATCH // NCORES          # batch rows per core = 4
L = 16                         # steps per chunk
C = SEQ // L                   # chunks = 128
NKS = 7                        # log2(C) Kogge-Stone passes
POS = SEQ + 16                 # xfull positions per b (pos 0..2048, padded)
F32R = mybir.dt.float32r
F32 = mybir.dt.float32

# wpack chunk layout (each chunk = 256 fp32 cols per partition)
#   0-1   : W_v   = B.T          (ktiles 0,1)
#   2-3   : W_scan= A.T
#   4-17  : W_carry[s] s=0..6    (2 ktiles each)
#   18-21 : W_o   = [C.T; D.T]   (4 ktiles, K=512)
NW_CHUNKS = 22
WP_COLS = NW_CHUNKS * 256 + 8  # + 8 cols: x0T ptile0 (4), ptile1 (4)


def _round_f32r(x):
    xb = np.ascontiguousarray(x, np.float32).view(np.uint32)
    half = np.uint32(1 << 11)
    mask = np.uint32(0xFFFFFFFF) << np.uint32(12)
    return ((xb + half) & mask).view(np.float32)


def _set_param(gamma, alpha, X11_params, X22_params, Skew_params, X21, C_, Dt):
    """float64 port of L2RU.set_param; returns A, B, D."""
    dtype = np.float64
    n = C_.shape[0]
    ID = np.eye(n, dtype=dtype)
    tril = np.tril_indices(n)
    triu = np.triu_indices(n, k=1)
    X11 = np.zeros((n, n), dtype); X11[tril] = X11_params
    X22 = np.zeros((n, n), dtype); X22[tril] = X22_params
    Sk = np.zeros((n, n), dtype)
    Sk[triu] = Skew_params
    Sk[(triu[1], triu[0])] = -Skew_params
    X21 = X21.astype(dtype); C_ = C_.astype(dtype); Dt = Dt.astype(dtype)

    Qm = (ID - Sk) @ np.linalg.inv(ID + Sk)
    eps_fac = 1.0  # exp(0.0)
    Z = X21 @ X21.T + X22 @ X22.T + Dt.T @ Dt + eps_fac * ID
    sig = 1.0 / (1.0 + np.exp(-float(alpha)))
    beta = float(gamma) ** 2 * sig / np.linalg.norm(Z, ord=2)
    H11 = X11 @ X11.T + C_.T @ C_ + beta * eps_fac * ID
    H12 = np.sqrt(beta) * (X11 @ X21.T + C_.T @ Dt)
    V = Z * beta - float(gamma) ** 2 * ID
    S = np.linalg.solve(V.T, H12.T)
    R = H12 @ S
    R = 0.5 * (R + R.T)
    negR = -R + 1e-6 * ID
    CR = np.linalg.cholesky(negR)
    CRH = np.linalg.cholesky(negR + H11)
    A = np.linalg.inv(CRH).T @ Qm @ CR.T
    B = A @ np.linalg.solve(H12.T, V.T)
    D = np.sqrt(beta) * Dt
    return A, B, D


def _build_program():
    nc = bacc.Bacc("TRN2", target_bir_lowering=False)
    ut_d = nc.dram_tensor("ut", [2, 128, BPC, SEQ], F32R, kind="ExternalInput").ap()
    wp_d = nc.dram_tensor("wp", [128, WP_COLS], F32R, kind="ExternalInput").ap()
    wf_d = nc.dram_tensor("wf", [L, 128, 2, 256], F32R, kind="ExternalInput").ap()
    xt_d = nc.dram_tensor("xt", [2, 128, BPC, SEQ], F32, kind="ExternalOutput").ap()
    ot_d = nc.dram_tensor("ot", [2, 128, BPC, SEQ], F32, kind="ExternalOutput").ap()

    with tile.TileContext(nc) as tc:
        _emit(nc, tc, ut_d, wp_d, wf_d, xt_d, ot_d)
    nc.compile()
    return nc


def _emit(nc, tc, ut_d, wp_d, wf_d, xt_d, ot_d):
    from contextlib import ExitStack
    ctx = ExitStack()
    with ctx:
        consts = ctx.enter_context(tc.tile_pool(name="consts", bufs=1))
        data = ctx.enter_context(tc.tile_pool(name="data", bufs=1))
        fixp = ctx.enter_context(tc.tile_pool(name="fixw", bufs=3))
        stage = ctx.enter_context(tc.tile_pool(name="stage", bufs=2))
        psum = ctx.enter_context(tc.tile_pool(name="psum", bufs=2, space="PSUM"))

        WP = consts.tile([128, WP_COLS], F32R, tag="wp")
        nc.sync.dma_start(out=WP[:], in_=wp_d[:])

        def lhsT(chunk, m):
            base = chunk * 256 + 128 * m
            return WP[:, base : base + 128]

        UT = []
        for p in range(2):
            t = data.tile([128, BPC, SEQ], F32R, tag=f"ut{p}")
            nc.sync.dma_start(out=t[:], in_=ut_d[p])
            UT.append(t)
        XF = [data.tile([128, BPC, POS], F32R, tag=f"xf{p}", name=f"xf{p}")
              for p in range(2)]
        # 4-D view: position 16*c + j  ->  [:, :, c, j]
        XF4 = [x[:].rearrange("p b (c j) -> p b c j", j=16) for x in XF]

        # ---- P1: V^T = B @ U^T  into XF positions 1..2048 --------------
        for m in range(2):
            for b in range(BPC):
                for blk in range(SEQ // 512):
                    ps = psum.tile([128, 512], F32, tag=f"ps{m}")
                    for k in range(2):
                        nc.tensor.matmul(ps[:], lhsT(0 + k, m),
                                         UT[k][:, b, 512 * blk : 512 * (blk + 1)],
                                         start=(k == 0), stop=(k == 1))
                    nc.vector.tensor_copy(
                        XF[m][:, b, 1 + 512 * blk : 1 + 512 * (blk + 1)], ps[:])

        # x0 passthrough at pos 0 and fold x0 @ A^T into pos 1 of chunk 0
        x0base = NW_CHUNKS * 256
        for p in range(2):
            nc.vector.tensor_copy(XF[p][:, :, 0],
                                  WP[:, x0base + 4 * p : x0base + 4 * (p + 1)])
        for m in range(2):
            ps0 = psum.tile([128, 512], F32, tag=f"ps{m}")
            for k in range(2):
                nc.tensor.matmul(ps0[:, 0:BPC], lhsT(2 + k, m),
                                 WP[:, x0base + 4 * k : x0base + 4 * (k + 1)],
                                 start=(k == 0), stop=(k == 1))
            nc.vector.tensor_add(XF[m][:, :, 1], ps0[:, 0:BPC], XF[m][:, :, 1])

        # ---- P2: local scans (15 steps over all chunks at once) --------
        # step j: x[16c+j+1] = x[16c+j] @ A.T + v  (v already at 16c+j+1)
        for j in range(1, L):
            rhs = [XF4[k][:, :, 0:C, j] for k in range(2)]
            if j + 1 < L:
                dsts = [XF4[m][:, :, 0:C, j + 1] for m in range(2)]
            else:
                dsts = [XF4[m][:, :, 1 : C + 1, 0] for m in range(2)]
            for m in range(2):
                ps = psum.tile([128, BPC, C], F32, tag=f"ps{m}")
                for k in range(2):
                    nc.tensor.matmul(ps[:], lhsT(2 + k, m), rhs[k],
                                     start=(k == 0), stop=(k == 1))
                nc.vector.tensor_add(dsts[m], ps[:], dsts[m])

        # ---- P3: Kogge-Stone carries over chunk-final states ------------
        G = []
        for p in range(2):
            g = data.tile([128, BPC, C], F32R, tag=f"g{p}")
            nc.vector.tensor_copy(g[:], XF4[p][:, :, 1 : C + 1, 0])
            G.append(g)
        for s in range(NKS):
            sh = 1 << s
            nf = C - sh
            # fp32r needs an even inner free count: matmul the full 128-chunk
            # width, DVE-add only the nf valid columns.  Both m-halves must
            # matmul the OLD G before either half's add updates it.
            pss = []
            for m in range(2):
                ps = psum.tile([128, BPC, C], F32, tag=f"ps{m}")
                for k in range(2):
                    nc.tensor.matmul(ps[:], lhsT(4 + 2 * s + k, m),
                                     G[k][:], start=(k == 0), stop=(k == 1))
                pss.append(ps)
            for m in range(2):
                dst = G[m][:, :, sh:]
                nc.vector.tensor_add(dst, pss[m][:, :, :nf], dst)

        # ---- P4: fix-up  x[16c+j] += s_c @ A^j   (c >= 1, j = 1..16) ----
        # s_c = G[:, :, c-1]  (global state at end of chunk c-1)
        for j in range(1, L + 1):
            wf = fixp.tile([128, 2, 256], F32R, tag="wf")
            nc.sync.dma_start(out=wf[:], in_=wf_d[j - 1])
            if j < L:
                dsts = [XF4[m][:, :, 1:C, j] for m in range(2)]
            else:
                dsts = [XF4[m][:, :, 2 : C + 1, 0] for m in range(2)]
            for m in range(2):
                ps = psum.tile([128, BPC, C], F32, tag=f"ps{m}")
                for k in range(2):
                    nc.tensor.matmul(ps[:],
                                     wf[:, k, 128 * m : 128 * m + 128],
                                     G[k][:], start=(k == 0), stop=(k == 1))
                nc.vector.tensor_add(dsts[m], ps[:, :, : C - 1], dsts[m])

        # ---- states out -------------------------------------------------
        for p in range(2):
            nc.sync.dma_start(out=xt_d[p],
                              in_=XF[p][:, :, 1 : 1 + SEQ].bitcast(F32))

        # ---- P5: outputs O^T = C @ X^T + D @ U^T ------------------------
        for m in range(2):
            for b in range(BPC):
                ob = stage.tile([128, SEQ], F32, tag="ob")
                for blk in range(SEQ // 512):
                    ps = psum.tile([128, 512], F32, tag=f"ps{m}")
                    for k in range(2):
                        nc.tensor.matmul(ps[:], lhsT(18 + k, m),
                                         XF[k][:, b, 512 * blk : 512 * (blk + 1)],
                                         start=(k == 0), stop=False)
                    for k in range(2):
                        nc.tensor.matmul(ps[:], lhsT(20 + k, m),
                                         UT[k][:, b, 512 * blk : 512 * (blk + 1)],
                                         start=False, stop=(k == 1))
                    nc.vector.tensor_copy(ob[:, 512 * blk : 512 * (blk + 1)], ps[:])
                nc.sync.dma_start(out=ot_d[m, :, b, :], in_=ob[:])


_PROGRAM_CACHE = {}
LAST_RUN = []
LAST_IN_MAPS = None


def _get_program():
    if "nc" not in _PROGRAM_CACHE:
        _PROGRAM_CACHE["nc"] = _build_program()
    return _PROGRAM_CACHE["nc"]


def kernel(input, state, gamma, alpha, X11_params, X22_params, Skew_params,
           X21, C, Dt):
    input = np.asarray(input, np.float32)
    state = np.asarray(state, np.float32)
    A64, B64, D64 = _set_param(np.asarray(gamma), np.asarray(alpha),
                               np.asarray(X11_params), np.asarray(X22_params),
                               np.asarray(Skew_params), np.asarray(X21),
                               np.asarray(C), np.asarray(Dt))
    AT = A64.T
    Cm = np.asarray(C, np.float64)

    # weight pack
    chunks = []
    for W in (B64.T, AT):
        chunks += [W[:128], W[128:]]
    P = AT.copy()
    carry_pows = []
    for s in range(NKS + 0):
        Pl = np.linalg.matrix_power(AT, L * (1 << s))
        carry_pows.append(Pl)
        chunks += [Pl[:128], Pl[128:]]
    WO = np.concatenate([Cm.T, D64.T], axis=0)  # [512, 256]
    chunks += [WO[i * 128 : (i + 1) * 128] for i in range(4)]
    wp = np.zeros((128, WP_COLS), np.float32)
    for i, ch in enumerate(chunks):
        wp[:, i * 256 : (i + 1) * 256] = ch.astype(np.float32)
    # x0^T per-core slot filled per core below
    wf = np.zeros((L, 128, 2, 256), np.float32)
    for j in range(1, L + 1):
        Pj = np.linalg.matrix_power(AT, j).astype(np.float32)
        wf[j - 1, :, 0] = Pj[:128]
        wf[j - 1, :, 1] = Pj[128:]
    wf = _round_f32r(wf)

    nc = _get_program()
    in_maps = []
    for c in range(NCORES):
        Uc = input[BPC * c : BPC * (c + 1)]              # [4, 2048, 256]
        UtT = np.ascontiguousarray(Uc.transpose(2, 0, 1))  # [256, 4, 2048]
        ut = _round_f32r(UtT).reshape(2, 128, BPC, SEQ)
        x0T = state[BPC * c : BPC * (c + 1)].T           # [256, 4]
        wpc = wp.copy()
        wpc[:, NW_CHUNKS * 256 : NW_CHUNKS * 256 + 4] = x0T[:128]
        wpc[:, NW_CHUNKS * 256 + 4 :] = x0T[128:]
        in_maps.append({"ut": ut, "wp": _round_f32r(wpc), "wf": wf})

    global LAST_IN_MAPS
    LAST_IN_MAPS = in_maps
    res = run_bass_kernel_spmd(nc, in_maps, list(range(NCORES)))
    LAST_RUN.append(res)

    outputs = np.empty((BATCH, SEQ, N), np.float32)
    states = np.empty((BATCH, SEQ + 1, N), np.float32)
    states[:, 0, :] = state
    for c in range(NCORES):
        ot = res.results[c]["ot"].reshape(N, BPC, SEQ)
        xt = res.results[c]["xt"].reshape(N, BPC, SEQ)
        outputs[BPC * c : BPC * (c + 1)] = ot.transpose(1, 2, 0)
        states[BPC * c : BPC * (c + 1), 1:] = xt.transpose(1, 2, 0)
    return outputs, states
